# revision 15
# baseline (speedup 1.0000x reference)
"""Trainium2 Bass kernel for nn_AxisSimplestSpline (center-anchored ramp basis,
col-tiled PE).

Math (per batch b, axis a), with g = (f - mins)/dx in [0,17):
  est_a(g) = V8 + sum_{k=8..16} a_k * relu(g-k) + sum_{k=1..8} c_k * min(g-k, 0)
  out[c]   = sum_a pinv[a,c] * est_a  (+ bias, added on host)

v3 over v2: the 17 ramp matmuls write only 48 of 128 output partitions, so
they are packed pairwise into disjoint PE column groups via tile_position —
chain A accumulates 9 ramps into PSUM partitions 0:48 (col groups 0-1),
chain B 8 ramps into partitions 64:112 (groups 2-3).  Paired MMs run
concurrently in the array (Δstart ~4ns), halving ramp streaming time.
The two PSUM halves go to DRAM by direct DMA (no drain op) and the host
sums A + B + bias.

Features now come from a single fp16 G16 = g - 8.5 tile (ACT) so DVE runs
dual-op tensor_scalar at 4x (~327ns/feature); only L1, L2, R16 — whose
slopes are boundary-sized — stay on the exact fp32 ACT path.
Numpy-simulated rel err 2.0e-3 (tolerance 2e-2).

Engine budget per [128,1024] tile:  PE ~4.7us (22 serial MM slots),
ACT ~4.2us (G16 + 3 features), DVE ~4.6us (14 features).
"""

import sys

sys.path.insert(0, "/opt/trn_rl_repo")

import numpy as np

import concourse.bacc as bacc
import concourse.mybir as mybir
import concourse.tile as tile
from concourse.bass_utils import run_bass_kernel_spmd

F32 = mybir.dt.float32
F16 = mybir.dt.float16
EPS = 1e-4
B, C, H, W = 8, 3, 1024, 1024
HW = H * W
NA, K = 8, 16
J = 16
NJ = HW // J
FREE = 1024
NSUP = NJ // FREE
NCH = FREE // 512
GC = 8.5  # G16 centering

# production order: GPSIMD (slow) and ACT exact feats first, then DVE feats
PROD = (
    [("L", 2, "GP"), ("L", 3, "GP"), ("L", 8, "GP")]
    + [("L", 1, "ACT"), ("R", 16, "ACT")]
    + [("R", k, "DVE") for k in range(8, 16)]
    + [("L", k, "DVE") for k in range(4, 8)]
)
NF = len(PROD)  # 17
# MM i consumes PROD[i]; chain A = even i (9 MMs, psum parts 0:48),
# chain B = odd i (8 MMs, parts 64:112)
CHAIN = ["A" if i % 2 == 0 else "B" for i in range(NF)]

_NC_CACHE = {}


def _build_nc():
    nc = bacc.Bacc(None, target_bir_lowering=False, debug=False)
    rawh_t = nc.dram_tensor("rawh", [C, HW], F16, kind="ExternalInput")
    rawl_t = nc.dram_tensor("rawl", [C, HW], F16, kind="ExternalInput")
    # par cols: 0 = -m-GC (G16 bias); 1 = m+1 (ACT-L k=1, scale=-1);
    # 3 = -m-16 (ACT-R k=16)
    par_t = nc.dram_tensor("par", [128, 4], F32, kind="ExternalInput")
    wfh_t = nc.dram_tensor("wfh", [C * J, 128], F16, kind="ExternalInput")
    wf2_t = nc.dram_tensor("wf2", [2 * C * J, 128], F16, kind="ExternalInput")
    wks_t = nc.dram_tensor("wks", [128, NF * C * J], F16, kind="ExternalInput")
    outa_t = nc.dram_tensor("outa", [C, HW], F32, kind="ExternalOutput")
    outb_t = nc.dram_tensor("outb", [C, HW], F32, kind="ExternalOutput")

    Relu = mybir.ActivationFunctionType.Relu
    Ident = mybir.ActivationFunctionType.Identity
    sub = mybir.AluOpType.subtract
    mx = mybir.AluOpType.max
    mn = mybir.AluOpType.min

    with tile.TileContext(nc) as tc:
        with (
            tc.tile_pool(name="const", bufs=1) as cpool,
            tc.tile_pool(name="io", bufs=3) as iopool,
            tc.tile_pool(name="g16", bufs=3) as gpool,
            tc.tile_pool(name="ff", bufs=22) as fpool,
            tc.tile_pool(name="ob", bufs=4) as obpool,
            tc.tile_pool(name="pf", bufs=2, space="PSUM") as pfpool,
            tc.tile_pool(name="po", bufs=2, space="PSUM") as popool,
        ):
            pT = cpool.tile([128, 4], F32)
            nc.sync.dma_start(out=pT[:], in_=par_t[:])
            wfh = cpool.tile([C * J, 128], F16)
            nc.sync.dma_start(out=wfh[:], in_=wfh_t[:])
            wf2 = cpool.tile([2 * C * J, 128], F16)
            nc.sync.dma_start(out=wf2[:], in_=wf2_t[:])
            wks = cpool.tile([128, NF * C * J], F16)
            nc.sync.dma_start(out=wks[:], in_=wks_t[:])

            rawh_v = rawh_t.ap().rearrange("c (j n) -> (c j) n", j=J)
            rawl_v = rawl_t.ap().rearrange("c (j n) -> (c j) n", j=J)
            outa_v = outa_t.ap().rearrange("c (j n) -> (c j) n", j=J)
            outb_v = outb_t.ap().rearrange("c (j n) -> (c j) n", j=J)

            fps = [None] * NSUP
            ops = [None] * NSUP

            def drain(n):
                o = ops[n]
                n0 = n * FREE
                # drains: DMA cannot read PSUM, so copy to SBUF (ACT + DVE)
                oba = obpool.tile([C * J, FREE], F32, tag="oba")
                nc.scalar.activation(oba[:], o[0 : C * J], Ident, scale=1.0)
                nc.sync.dma_start(out=outa_v[:, n0 : n0 + FREE], in_=oba[:])
                obb = obpool.tile([C * J, FREE], F32, tag="obb")
                nc.vector.tensor_scalar(
                    out=obb[:], in0=o[64 : 64 + C * J], scalar1=0.0,
                    scalar2=None, op0=mybir.AluOpType.add,
                )
                nc.sync.dma_start(out=outb_v[:, n0 : n0 + FREE], in_=obb[:])

            def load_and_project(n):
                n0 = n * FREE
                r = iopool.tile([2 * C * J, FREE], F16, tag="rhs")
                nc.sync.dma_start(out=r[: C * J], in_=rawh_v[:, n0 : n0 + FREE])
                nc.sync.dma_start(out=r[C * J :], in_=rawl_v[:, n0 : n0 + FREE])
                f = pfpool.tile([128, FREE], F32, tag="fps")
                for h in range(NCH):
                    sl = slice(h * 512, (h + 1) * 512)
                    nc.tensor.matmul(f[:, sl], wfh[:], r[: C * J, sl], start=True, stop=False)
                    nc.tensor.matmul(f[:, sl], wf2[:], r[:, sl], start=False, stop=True)
                fps[n] = f

            load_and_project(0)

            for n in range(NSUP):
                if n + 1 < NSUP:
                    load_and_project(n + 1)

                f = fps[n]
                g16 = gpool.tile([128, FREE], F16, tag="g16")
                nc.scalar.activation(g16[:], f[:], Ident, bias=pT[:, 0:1], scale=1.0)
                feats = [None] * NF

                def centered(eng, ft, side, k):
                    if side == "R":
                        mk = (17.0 - k) / 2.0
                        eng.tensor_scalar(
                            out=ft[:], in0=g16[:], scalar1=float(k - GC + mk),
                            scalar2=float(-mk), op0=sub, op1=mx,
                        )
                    else:
                        mk = k / 2.0
                        eng.tensor_scalar(
                            out=ft[:], in0=g16[:], scalar1=float(k - GC - mk),
                            scalar2=float(mk), op0=sub, op1=mn,
                        )

                for i, (side, k, eng) in enumerate(PROD):
                    if eng != "GP":
                        continue
                    ft = fpool.tile([128, FREE], F16, tag="F")
                    centered(nc.gpsimd, ft, side, k)
                    feats[i] = ft
                for i, (side, k, eng) in enumerate(PROD):
                    if eng != "ACT":
                        continue
                    ft = fpool.tile([128, FREE], F16, tag="F")
                    if side == "L":  # max(k-g,0) = Relu(-f + (m+k)); col 1
                        nc.scalar.activation(ft[:], f[:], Relu, bias=pT[:, k : k + 1], scale=-1.0)
                    else:  # R16: Relu(f - m - 16); col 3
                        nc.scalar.activation(ft[:], f[:], Relu, bias=pT[:, 3:4], scale=1.0)
                    feats[i] = ft
                for i, (side, k, eng) in enumerate(PROD):
                    if eng != "DVE":
                        continue
                    ft = fpool.tile([128, FREE], F16, tag="F")
                    centered(nc.vector, ft, side, k)
                    feats[i] = ft

                # previous tile's drains: emitted here so ACT/DVE program
                # order is [feats(n), drain(n-1)] and never blocks on knots(n)
                if n >= 1:
                    drain(n - 1)

                o = popool.tile([128, FREE], F32, tag="ops")
                na = CHAIN.count("A")
                nb = CHAIN.count("B")
                for h in range(NCH):
                    sl = slice(h * 512, (h + 1) * 512)
                    ia = ib = 0
                    for i in range(NF):
                        wk = wks[:, i * C * J : (i + 1) * C * J]
                        if CHAIN[i] == "A":
                            nc.tensor.matmul(
                                o[0 : C * J, sl], wk, feats[i][:, sl],
                                start=(ia == 0), stop=(ia == na - 1),
                                tile_position=(0, 0),
                            )
                            ia += 1
                        else:
                            nc.tensor.matmul(
                                o[64 : 64 + C * J, sl], wk, feats[i][:, sl],
                                start=(ib == 0), stop=(ib == nb - 1),
                                tile_position=(0, 64),
                            )
                            ib += 1
                ops[n] = o

            drain(NSUP - 1)
    nc.compile()
    return nc


def _host_params(raw, ys, A):
    in_maps = []
    jr = lambda x: np.repeat(x, J)
    for b in range(B):
        Ab = A[b].astype(np.float32)
        mins = np.minimum(Ab, 0).sum(axis=0)
        maxs = np.maximum(Ab, 0).sum(axis=0)
        pinv = np.linalg.pinv(Ab).astype(np.float32)  # [8, 3]
        span = (maxs + np.float32(EPS) - mins).astype(np.float32)
        dx = (span / np.float32(K + 1)).astype(np.float32)
        inv_dx = (np.float32(1.0) / dx).astype(np.float32)
        Y = np.concatenate(
            [mins[:, None], ys[b].astype(np.float32), maxs[:, None]], axis=1
        )  # [8, 18]
        s = np.diff(Y, axis=1).astype(np.float32)  # [8, 17]

        a = np.zeros((NA, 17), np.float32)
        c = np.zeros((NA, 17), np.float32)
        a[:, 8] = s[:, 8]
        for k in range(9, 17):
            a[:, k] = s[:, k] - s[:, k - 1]
        c[:, 8] = s[:, 7]
        for k in range(1, 8):
            c[:, k] = s[:, k - 1] - s[:, k]

        m = (mins * inv_dx).astype(np.float32)
        par = np.zeros((128, 4), np.float32)
        par[:, 0] = jr(-m - np.float32(GC))
        par[:, 1] = jr(m + 1)
        par[:, 2] = jr(m + 2)
        par[:, 3] = jr(-m - 16)

        wf = (Ab * inv_dx[None, :]).astype(np.float32)  # [3, 8]
        wfm = np.zeros((C * J, 128), np.float32)
        for jj in range(J):
            for cc in range(C):
                for aa in range(NA):
                    wfm[cc * J + jj, aa * J + jj] = wf[cc, aa]
        wfh = wfm.astype(np.float16)
        wfl = (wfm - wfh.astype(np.float32)).astype(np.float16)
        wf2 = np.concatenate([wfl, wfh], axis=0)

        bias = Y[:, 8].astype(np.float32).copy()  # V8
        wco = np.zeros((NA, NF), np.float32)
        for i, (side, k, eng) in enumerate(PROD):
            if side == "R":
                wco[:, i] = a[:, k]
                if eng == "DVE":
                    bias += a[:, k] * np.float32((17.0 - k) / 2.0)
            else:
                if eng == "ACT":
                    wco[:, i] = -c[:, k]
                else:
                    wco[:, i] = c[:, k]
                    bias -= c[:, k] * np.float32(k / 2.0)

        wks = np.zeros((128, NF * C * J), np.float16)
        for i in range(NF):
            for jj in range(J):
                for cc in range(C):
                    for aa in range(NA):
                        wks[aa * J + jj, i * C * J + cc * J + jj] = (
                            pinv[aa, cc] * wco[aa, i]
                        )

        b0 = (pinv * bias[:, None]).sum(axis=0)  # [3], added on host

        rb = np.ascontiguousarray(raw[b].reshape(C, HW), np.float32)
        rh = rb.astype(np.float16)
        rl = (rb - rh.astype(np.float32)).astype(np.float16)
        in_maps.append(
            {
                "rawh": rh,
                "rawl": rl,
                "par": par,
                "wfh": wfh,
                "wf2": wf2,
                "wks": wks,
                "_b0": b0,  # host-side only
            }
        )
    return in_maps


def kernel(raw, ys, A):
    raw = np.asarray(raw, np.float32)
    ys = np.asarray(ys, np.float32)
    A = np.asarray(A, np.float32)
    if "nc" not in _NC_CACHE:
        _NC_CACHE["nc"] = _build_nc()
    nc = _NC_CACHE["nc"]
    in_maps = _host_params(raw, ys, A)
    dev_maps = [{k: v for k, v in im.items() if not k.startswith("_")} for im in in_maps]
    res = run_bass_kernel_spmd(nc, dev_maps, core_ids=list(range(B)))
    outs = []
    for b in range(B):
        oa = res.results[b]["outa"].astype(np.float32)
        ob = res.results[b]["outb"].astype(np.float32)
        o = oa + ob + in_maps[b]["_b0"][:, None].astype(np.float32)
        outs.append(o.reshape(C, H, W))
    return np.stack(outs).astype(np.float32)


# revision 17
# speedup vs baseline: 7.7867x; 7.7867x over previous
"""Trainium2 Bass kernel for nn_AxisSimplestSpline (center-anchored ramp basis,
col-tiled PE).

Math (per batch b, axis a), with g = (f - mins)/dx in [0,17):
  est_a(g) = V8 + sum_{k=8..16} a_k * relu(g-k) + sum_{k=1..8} c_k * min(g-k, 0)
  out[c]   = sum_a pinv[a,c] * est_a  (+ bias, added on host)

v3 over v2: the 17 ramp matmuls write only 48 of 128 output partitions, so
they are packed pairwise into disjoint PE column groups via tile_position —
chain A accumulates 9 ramps into PSUM partitions 0:48 (col groups 0-1),
chain B 8 ramps into partitions 64:112 (groups 2-3).  Paired MMs run
concurrently in the array (Δstart ~4ns), halving ramp streaming time.
The two PSUM halves go to DRAM by direct DMA (no drain op) and the host
sums A + B + bias.

Features now come from a single fp16 G16 = g - 8.5 tile (ACT) so DVE runs
dual-op tensor_scalar at 4x (~327ns/feature); only L1, L2, R16 — whose
slopes are boundary-sized — stay on the exact fp32 ACT path.
Numpy-simulated rel err 2.0e-3 (tolerance 2e-2).

Engine budget per [128,1024] tile:  PE ~4.7us (22 serial MM slots),
ACT ~4.2us (G16 + 3 features), DVE ~4.6us (14 features).
"""

import sys

sys.path.insert(0, "/opt/trn_rl_repo")

import numpy as np

import concourse.bacc as bacc
import concourse.mybir as mybir
import concourse.tile as tile
from concourse.bass_utils import run_bass_kernel_spmd

F32 = mybir.dt.float32
F16 = mybir.dt.float16
EPS = 1e-4
B, C, H, W = 8, 3, 1024, 1024
HW = H * W
NA, K = 8, 16
J = 16
NJ = HW // J
FREE = 1024
NSUP = NJ // FREE
NCH = FREE // 512
GC = 8.5  # G16 centering

# production order: ACT exact feats first, then DVE feats
# (GPSIMD tensor_scalar measured 14.7us per [128,1024] op — unusable)
PROD = (
    [("L", 1, "ACT"), ("L", 2, "ACT"), ("R", 16, "ACT")]
    + [("R", k, "DVE") for k in range(8, 16)]
    + [("L", k, "DVE") for k in range(3, 9)]
)
NF = len(PROD)  # 17
# MM i consumes PROD[i]; chain A = even i (9 MMs, psum parts 0:48),
# chain B = odd i (8 MMs, parts 64:112)
CHAIN = ["A" if i % 2 == 0 else "B" for i in range(NF)]

_NC_CACHE = {}


def _build_nc():
    nc = bacc.Bacc(None, target_bir_lowering=False, debug=False)
    rawh_t = nc.dram_tensor("rawh", [C, HW], F16, kind="ExternalInput")
    rawl_t = nc.dram_tensor("rawl", [C, HW], F16, kind="ExternalInput")
    # par cols: 0 = -m-GC (G16 bias); 1 = m+1 (ACT-L k=1, scale=-1);
    # 3 = -m-16 (ACT-R k=16)
    par_t = nc.dram_tensor("par", [128, 4], F32, kind="ExternalInput")
    wfh_t = nc.dram_tensor("wfh", [C * J, 128], F16, kind="ExternalInput")
    wf2_t = nc.dram_tensor("wf2", [2 * C * J, 128], F16, kind="ExternalInput")
    wks_t = nc.dram_tensor("wks", [128, NF * C * J], F16, kind="ExternalInput")
    outa_t = nc.dram_tensor("outa", [C, HW], F32, kind="ExternalOutput")
    outb_t = nc.dram_tensor("outb", [C, HW], F32, kind="ExternalOutput")

    Relu = mybir.ActivationFunctionType.Relu
    Ident = mybir.ActivationFunctionType.Identity
    sub = mybir.AluOpType.subtract
    mx = mybir.AluOpType.max
    mn = mybir.AluOpType.min

    with tile.TileContext(nc) as tc:
        with (
            tc.tile_pool(name="const", bufs=1) as cpool,
            tc.tile_pool(name="io", bufs=3) as iopool,
            tc.tile_pool(name="g16", bufs=3) as gpool,
            tc.tile_pool(name="ff", bufs=22) as fpool,
            tc.tile_pool(name="ob", bufs=4) as obpool,
            tc.tile_pool(name="pf", bufs=2, space="PSUM") as pfpool,
            tc.tile_pool(name="po", bufs=2, space="PSUM") as popool,
        ):
            pT = cpool.tile([128, 4], F32)
            nc.sync.dma_start(out=pT[:], in_=par_t[:])
            wfh = cpool.tile([C * J, 128], F16)
            nc.sync.dma_start(out=wfh[:], in_=wfh_t[:])
            wf2 = cpool.tile([2 * C * J, 128], F16)
            nc.sync.dma_start(out=wf2[:], in_=wf2_t[:])
            wks = cpool.tile([128, NF * C * J], F16)
            nc.sync.dma_start(out=wks[:], in_=wks_t[:])

            rawh_v = rawh_t.ap().rearrange("c (j n) -> (c j) n", j=J)
            rawl_v = rawl_t.ap().rearrange("c (j n) -> (c j) n", j=J)
            outa_v = outa_t.ap().rearrange("c (j n) -> (c j) n", j=J)
            outb_v = outb_t.ap().rearrange("c (j n) -> (c j) n", j=J)

            fps = [None] * NSUP
            ops = [None] * NSUP

            def drain(n):
                o = ops[n]
                n0 = n * FREE
                # drains: DMA cannot read PSUM, so copy to SBUF (ACT + DVE)
                oba = obpool.tile([C * J, FREE], F32, tag="oba")
                nc.scalar.activation(oba[:], o[0 : C * J], Ident, scale=1.0)
                nc.sync.dma_start(out=outa_v[:, n0 : n0 + FREE], in_=oba[:])
                obb = obpool.tile([C * J, FREE], F32, tag="obb")
                nc.vector.tensor_scalar(
                    out=obb[:], in0=o[64 : 64 + C * J], scalar1=0.0,
                    scalar2=None, op0=mybir.AluOpType.add,
                )
                nc.sync.dma_start(out=outb_v[:, n0 : n0 + FREE], in_=obb[:])

            def load_and_project(n):
                n0 = n * FREE
                r = iopool.tile([2 * C * J, FREE], F16, tag="rhs")
                nc.sync.dma_start(out=r[: C * J], in_=rawh_v[:, n0 : n0 + FREE])
                nc.sync.dma_start(out=r[C * J :], in_=rawl_v[:, n0 : n0 + FREE])
                f = pfpool.tile([128, FREE], F32, tag="fps")
                for h in range(NCH):
                    sl = slice(h * 512, (h + 1) * 512)
                    nc.tensor.matmul(f[:, sl], wfh[:], r[: C * J, sl], start=True, stop=False)
                    nc.tensor.matmul(f[:, sl], wf2[:], r[:, sl], start=False, stop=True)
                fps[n] = f

            load_and_project(0)

            for n in range(NSUP):
                if n + 1 < NSUP:
                    load_and_project(n + 1)

                f = fps[n]
                g16 = gpool.tile([128, FREE], F16, tag="g16")
                nc.scalar.activation(g16[:], f[:], Ident, bias=pT[:, 0:1], scale=1.0)
                feats = [None] * NF

                def centered(eng, ft, side, k):
                    if side == "R":
                        mk = (17.0 - k) / 2.0
                        eng.tensor_scalar(
                            out=ft[:], in0=g16[:], scalar1=float(k - GC + mk),
                            scalar2=float(-mk), op0=sub, op1=mx,
                        )
                    else:
                        mk = k / 2.0
                        eng.tensor_scalar(
                            out=ft[:], in0=g16[:], scalar1=float(k - GC - mk),
                            scalar2=float(mk), op0=sub, op1=mn,
                        )

                for i, (side, k, eng) in enumerate(PROD):
                    if eng != "ACT":
                        continue
                    ft = fpool.tile([128, FREE], F16, tag="F")
                    if side == "L":  # max(k-g,0) = Relu(-f + (m+k)); col 1
                        nc.scalar.activation(ft[:], f[:], Relu, bias=pT[:, k : k + 1], scale=-1.0)
                    else:  # R16: Relu(f - m - 16); col 3
                        nc.scalar.activation(ft[:], f[:], Relu, bias=pT[:, 3:4], scale=1.0)
                    feats[i] = ft
                for i, (side, k, eng) in enumerate(PROD):
                    if eng != "DVE":
                        continue
                    ft = fpool.tile([128, FREE], F16, tag="F")
                    centered(nc.vector, ft, side, k)
                    feats[i] = ft

                # previous tile's drains: emitted here so ACT/DVE program
                # order is [feats(n), drain(n-1)] and never blocks on knots(n)
                if n >= 1:
                    drain(n - 1)

                o = popool.tile([128, FREE], F32, tag="ops")
                na = CHAIN.count("A")
                nb = CHAIN.count("B")
                for h in range(NCH):
                    sl = slice(h * 512, (h + 1) * 512)
                    ia = ib = 0
                    for i in range(NF):
                        wk = wks[:, i * C * J : (i + 1) * C * J]
                        if CHAIN[i] == "A":
                            nc.tensor.matmul(
                                o[0 : C * J, sl], wk, feats[i][:, sl],
                                start=(ia == 0), stop=(ia == na - 1),
                                tile_position=(0, 0),
                            )
                            ia += 1
                        else:
                            nc.tensor.matmul(
                                o[64 : 64 + C * J, sl], wk, feats[i][:, sl],
                                start=(ib == 0), stop=(ib == nb - 1),
                                tile_position=(0, 64),
                            )
                            ib += 1
                ops[n] = o

            drain(NSUP - 1)
    nc.compile()
    return nc


def _host_params(raw, ys, A):
    in_maps = []
    jr = lambda x: np.repeat(x, J)
    for b in range(B):
        Ab = A[b].astype(np.float32)
        mins = np.minimum(Ab, 0).sum(axis=0)
        maxs = np.maximum(Ab, 0).sum(axis=0)
        pinv = np.linalg.pinv(Ab).astype(np.float32)  # [8, 3]
        span = (maxs + np.float32(EPS) - mins).astype(np.float32)
        dx = (span / np.float32(K + 1)).astype(np.float32)
        inv_dx = (np.float32(1.0) / dx).astype(np.float32)
        Y = np.concatenate(
            [mins[:, None], ys[b].astype(np.float32), maxs[:, None]], axis=1
        )  # [8, 18]
        s = np.diff(Y, axis=1).astype(np.float32)  # [8, 17]

        a = np.zeros((NA, 17), np.float32)
        c = np.zeros((NA, 17), np.float32)
        a[:, 8] = s[:, 8]
        for k in range(9, 17):
            a[:, k] = s[:, k] - s[:, k - 1]
        c[:, 8] = s[:, 7]
        for k in range(1, 8):
            c[:, k] = s[:, k - 1] - s[:, k]

        m = (mins * inv_dx).astype(np.float32)
        par = np.zeros((128, 4), np.float32)
        par[:, 0] = jr(-m - np.float32(GC))
        par[:, 1] = jr(m + 1)
        par[:, 2] = jr(m + 2)
        par[:, 3] = jr(-m - 16)

        wf = (Ab * inv_dx[None, :]).astype(np.float32)  # [3, 8]
        wfm = np.zeros((C * J, 128), np.float32)
        for jj in range(J):
            for cc in range(C):
                for aa in range(NA):
                    wfm[cc * J + jj, aa * J + jj] = wf[cc, aa]
        wfh = wfm.astype(np.float16)
        wfl = (wfm - wfh.astype(np.float32)).astype(np.float16)
        wf2 = np.concatenate([wfl, wfh], axis=0)

        bias = Y[:, 8].astype(np.float32).copy()  # V8
        wco = np.zeros((NA, NF), np.float32)
        for i, (side, k, eng) in enumerate(PROD):
            if side == "R":
                wco[:, i] = a[:, k]
                if eng == "DVE":
                    bias += a[:, k] * np.float32((17.0 - k) / 2.0)
            else:
                if eng == "ACT":
                    wco[:, i] = -c[:, k]
                else:
                    wco[:, i] = c[:, k]
                    bias -= c[:, k] * np.float32(k / 2.0)

        wks = np.zeros((128, NF * C * J), np.float16)
        for i in range(NF):
            for jj in range(J):
                for cc in range(C):
                    for aa in range(NA):
                        wks[aa * J + jj, i * C * J + cc * J + jj] = (
                            pinv[aa, cc] * wco[aa, i]
                        )

        b0 = (pinv * bias[:, None]).sum(axis=0)  # [3], added on host

        rb = np.ascontiguousarray(raw[b].reshape(C, HW), np.float32)
        rh = rb.astype(np.float16)
        rl = (rb - rh.astype(np.float32)).astype(np.float16)
        in_maps.append(
            {
                "rawh": rh,
                "rawl": rl,
                "par": par,
                "wfh": wfh,
                "wf2": wf2,
                "wks": wks,
                "_b0": b0,  # host-side only
            }
        )
    return in_maps


def kernel(raw, ys, A):
    raw = np.asarray(raw, np.float32)
    ys = np.asarray(ys, np.float32)
    A = np.asarray(A, np.float32)
    if "nc" not in _NC_CACHE:
        _NC_CACHE["nc"] = _build_nc()
    nc = _NC_CACHE["nc"]
    in_maps = _host_params(raw, ys, A)
    dev_maps = [{k: v for k, v in im.items() if not k.startswith("_")} for im in in_maps]
    res = run_bass_kernel_spmd(nc, dev_maps, core_ids=list(range(B)))
    outs = []
    for b in range(B):
        oa = res.results[b]["outa"].astype(np.float32)
        ob = res.results[b]["outb"].astype(np.float32)
        o = oa + ob + in_maps[b]["_b0"][:, None].astype(np.float32)
        outs.append(o.reshape(C, H, W))
    return np.stack(outs).astype(np.float32)
